# revision 5
# baseline (speedup 1.0000x reference)
import numpy as np

B, S, D, H = 16, 4096, 256, 256
NCORES = 8
BLOCAL = B // NCORES  # 2
SB = 256  # scan steps per superblock (bank cols = 2 chunks x SB = 512 fp32 = 1 bank)

_CACHE = {}


def _build(s_total=S, sb=SB, wdt_name="float32", has_bias=False):
    """Build the per-core SPMD bass program.

    Layout (per core, B_local=2):
      xte/xto [D, s_total]: col = 2*i + b for the i-th even/odd step, batch b.
      h0      [128, 2, 2]:  h0[p, k, b] = state0[b, k*128+p]  (h^T chunks).
      wx/wh   [256, 256]:   natural; lhsT quadrant = w[k*128:, m*128:].
      yt      [128, 2, 2*s_total]: yt[p, c, 2*s+b] = h_t[b, c*128+p].

    Per block: GEMM prefills xp^T into PSUM banks (start=True sets
    has_written), scan matmuls accumulate Wh^T @ h^T on top (start=False),
    one Tanh activation per step reads both chunks from one bank.
    """
    import concourse.bass as bass
    import concourse.tile as tile
    from concourse import bacc, mybir

    f32 = mybir.dt.float32
    wdt = getattr(mybir.dt, wdt_name)
    nblk = s_total // sb
    assert s_total % sb == 0 and sb % 2 == 0
    Tanh = mybir.ActivationFunctionType.Tanh
    PSUM = bass.MemorySpace.PSUM

    nc = bacc.Bacc("TRN2", target_bir_lowering=False, debug=False)
    xte_d = nc.dram_tensor("xte", [D, s_total], wdt, kind="ExternalInput")
    xto_d = nc.dram_tensor("xto", [D, s_total], wdt, kind="ExternalInput")
    h0_d = nc.dram_tensor("h0", [128, 2, 2], wdt, kind="ExternalInput")
    wx_d = nc.dram_tensor("wx", [D, H], wdt, kind="ExternalInput")
    wh_d = nc.dram_tensor("wh", [H, H], wdt, kind="ExternalInput")
    if has_bias:
        bias_d = nc.dram_tensor("bias", [1, H], wdt, kind="ExternalInput")
    yt_d = nc.dram_tensor("yt", [128, 2, 2 * s_total], wdt, kind="ExternalOutput")

    with tile.TileContext(nc) as tc:
        frees = []

        def T(shape, dt, name, space=None):
            kw = {"space": space} if space is not None else {}
            t, f = tc.tile(shape, dt, name=name, **kw)
            frees.append(f)
            return t

        wx_sb = T([128, 2, H], wdt, "wx_sb")
        wh_sb = T([128, 2, H], wdt, "wh_sb")
        h0_sb = T([128, 2, 2], wdt, "h0_sb")
        xe_sb = [T([128, 2, sb], wdt, f"xe{i}") for i in range(2)]
        xo_sb = [T([128, 2, sb], wdt, f"xo{i}") for i in range(2)]
        ht_sb = [T([128, 2, 2 * sb], wdt, f"ht{i}") for i in range(2)]
        banks = [
            [T([128, 2, sb], f32, f"pb{i}_{p}", space=PSUM) for p in range(2)]
            for i in range(2)
        ]
        if has_bias:
            bias_sb = T([1, H], wdt, "bias_sb")
            ones_sb = T([1, sb], wdt, "ones_sb")
            nc.sync.dma_start(bias_sb[:, :], bias_d[:, :])
            nc.gpsimd.memset(ones_sb[:, :], 1.0)

        for k in range(2):
            nc.sync.dma_start(wx_sb[:, k, :], wx_d[k * 128 : (k + 1) * 128, :])
            nc.sync.dma_start(wh_sb[:, k, :], wh_d[k * 128 : (k + 1) * 128, :])
        nc.sync.dma_start(h0_sb[:, :, :], h0_d[:, :, :])

        for blk in range(nblk):
            bi = blk % 2
            for k in range(2):
                nc.sync.dma_start(
                    xe_sb[bi][:, k, :],
                    xte_d[k * 128 : (k + 1) * 128, blk * sb : (blk + 1) * sb],
                )
                nc.sync.dma_start(
                    xo_sb[bi][:, k, :],
                    xto_d[k * 128 : (k + 1) * 128, blk * sb : (blk + 1) * sb],
                )

            # GEMM prefill: bank[p][:, m, n] = sum_d wx[d, m*128+.] * x[d, n]
            for p, xsb in ((0, xe_sb[bi]), (1, xo_sb[bi])):
                for m in range(2):
                    for k in range(2):
                        nc.tensor.matmul(
                            banks[bi][p][:, m, :],
                            wx_sb[:, k, m * 128 : (m + 1) * 128],
                            xsb[:, k, :],
                            start=(m == 0 and k == 0),
                            stop=False,
                            skip_group_check=True,
                        )
                    if has_bias:
                        nc.tensor.matmul(
                            banks[bi][p][:, m, :],
                            bias_sb[:, m * 128 : (m + 1) * 128],
                            ones_sb[:, :],
                            start=False,
                            stop=False,
                            skip_group_check=True,
                        )

            # serial scan: h_s = tanh(xp_s + Wh^T @ h_{s-1})  (all transposed)
            for s in range(sb):
                p = s & 1
                sc = s >> 1
                bank = banks[bi][p]
                for m in range(2):
                    for k in range(2):
                        if s == 0 and blk == 0:
                            hp = h0_sb[:, k, :]
                        elif s == 0:
                            hp = ht_sb[1 - bi][:, k, 2 * sb - 2 : 2 * sb]
                        else:
                            hp = ht_sb[bi][:, k, 2 * s - 2 : 2 * s]
                        nc.tensor.matmul(
                            bank[:, m, 2 * sc : 2 * sc + 2],
                            wh_sb[:, k, m * 128 : (m + 1) * 128],
                            hp,
                            start=False,
                            stop=(m == 1 and k == 1 and s >= sb - 2),
                            skip_group_check=True,
                        )
                nc.scalar.activation(
                    ht_sb[bi][:, :, 2 * s : 2 * s + 2],
                    bank[:, :, 2 * sc : 2 * sc + 2],
                    Tanh,
                    bias=0.0,
                    scale=1.0,
                )

            nc.gpsimd.dma_start(
                yt_d[:, :, blk * 2 * sb : (blk + 1) * 2 * sb], ht_sb[bi][:, :, :]
            )

        for f in reversed(frees):
            f()

    nc.compile()
    return nc


def _get_nc(s_total=S, sb=SB, wdt_name="float32", has_bias=False):
    key = (s_total, sb, wdt_name, has_bias)
    if key not in _CACHE:
        _CACHE[key] = _build(s_total, sb, wdt_name, has_bias)
    return _CACHE[key]


LAST_EXEC_NS = None
LAST_RESULTS = None


def _np_dt(wdt_name):
    if wdt_name == "bfloat16":
        import ml_dtypes

        return ml_dtypes.bfloat16
    return np.float32


def kernel(inputs, state0, Wx, Wh, b, s_total=S, sb=SB, wdt_name="float32", trace=False):
    global LAST_EXEC_NS, LAST_RESULTS
    from concourse.bass_utils import run_bass_kernel_spmd

    inputs = np.asarray(inputs, dtype=np.float32)
    state0 = np.asarray(state0, dtype=np.float32)
    Wx = np.asarray(Wx, dtype=np.float32)
    Wh = np.asarray(Wh, dtype=np.float32)
    b = np.asarray(b, dtype=np.float32)
    has_bias = bool(np.any(b != 0))
    ndt = _np_dt(wdt_name)

    nc = _get_nc(s_total, sb, wdt_name, has_bias)

    in_maps = []
    wx_c = np.ascontiguousarray(Wx, dtype=ndt)
    wh_c = np.ascontiguousarray(Wh, dtype=ndt)
    for c in range(NCORES):
        xc = inputs[BLOCAL * c : BLOCAL * (c + 1), :s_total]  # [2, s, D]
        xt = np.transpose(xc, (2, 1, 0))  # [D, s, 2]
        xte = np.ascontiguousarray(xt[:, 0::2, :].reshape(D, s_total), dtype=ndt)
        xto = np.ascontiguousarray(xt[:, 1::2, :].reshape(D, s_total), dtype=ndt)
        h0 = np.ascontiguousarray(
            np.transpose(state0[BLOCAL * c : BLOCAL * (c + 1)].reshape(2, 2, 128), (2, 1, 0)),
            dtype=ndt,
        )  # [128, 2(chunk), 2(batch)]
        m = {"xte": xte, "xto": xto, "h0": h0, "wx": wx_c, "wh": wh_c}
        if has_bias:
            m["bias"] = np.ascontiguousarray(b.reshape(1, H), dtype=ndt)
        in_maps.append(m)

    res = run_bass_kernel_spmd(nc, in_maps, core_ids=list(range(NCORES)), trace=trace)
    LAST_EXEC_NS = res.exec_time_ns
    LAST_RESULTS = res

    outs = []
    for c in range(NCORES):
        yt = np.asarray(res.results[c]["yt"], dtype=np.float32)  # [128, 2, 2s]
        y = yt.reshape(128, 2, s_total, 2)  # (p, ch, s, b)
        y = np.transpose(y, (3, 2, 1, 0)).reshape(BLOCAL, s_total, H)
        outs.append(y)
    return np.ascontiguousarray(np.concatenate(outs, axis=0), dtype=np.float32)


# revision 6
# speedup vs baseline: 7.5956x; 7.5956x over previous
import numpy as np

B, S, D, H = 16, 4096, 256, 256
NCORES = 8
BLOCAL = B // NCORES  # 2
SB = 256  # scan steps per superblock (bank cols = 2 chunks x SB = 512 fp32 = 1 bank)

_CACHE = {}


def _build(s_total=S, sb=SB, wdt_name="float32", has_bias=False):
    """Build the per-core SPMD bass program.

    Layout (per core, B_local=2):
      xte/xto [D, s_total]: col = 2*i + b for the i-th even/odd step, batch b.
      h0      [128, 2, 2]:  h0[p, k, b] = state0[b, k*128+p]  (h^T chunks).
      wx/wh   [256, 256]:   natural; lhsT quadrant = w[k*128:, m*128:].
      yt      [128, 2, 2*s_total]: yt[p, c, 2*s+b] = h_t[b, c*128+p].

    Per block: GEMM prefills xp^T into PSUM banks (start=True sets
    has_written), scan matmuls accumulate Wh^T @ h^T on top (start=False),
    one Tanh activation per step reads both chunks from one bank.
    """
    import concourse.bass as bass
    import concourse.tile as tile
    from concourse import bacc, mybir

    f32 = mybir.dt.float32
    wdt = getattr(mybir.dt, wdt_name)
    nblk = s_total // sb
    assert s_total % sb == 0 and sb % 2 == 0
    Tanh = mybir.ActivationFunctionType.Tanh
    PSUM = bass.MemorySpace.PSUM

    nc = bacc.Bacc("TRN2", target_bir_lowering=False, debug=False)
    xte_d = nc.dram_tensor("xte", [D, s_total], wdt, kind="ExternalInput")
    xto_d = nc.dram_tensor("xto", [D, s_total], wdt, kind="ExternalInput")
    h0_d = nc.dram_tensor("h0", [128, 2, 2], wdt, kind="ExternalInput")
    wx_d = nc.dram_tensor("wx", [D, H], wdt, kind="ExternalInput")
    wh_d = nc.dram_tensor("wh", [H, H], wdt, kind="ExternalInput")
    if has_bias:
        bias_d = nc.dram_tensor("bias", [1, H], wdt, kind="ExternalInput")
    yt_d = nc.dram_tensor("yt", [128, 2, 2 * s_total], wdt, kind="ExternalOutput")

    with tile.TileContext(nc) as tc:
        frees = []

        def T(shape, dt, name, space=None):
            kw = {"space": space} if space is not None else {}
            t, f = tc.tile(shape, dt, name=name, **kw)
            frees.append(f)
            return t

        wx_sb = T([128, 2, H], wdt, "wx_sb")
        wh_sb = T([128, 2, H], wdt, "wh_sb")
        h0_sb = T([128, 2, 2], wdt, "h0_sb")
        xe_sb = [T([128, 2, sb], wdt, f"xe{i}") for i in range(2)]
        xo_sb = [T([128, 2, sb], wdt, f"xo{i}") for i in range(2)]
        ht_sb = [T([128, 2, 2 * sb], wdt, f"ht{i}") for i in range(2)]
        banks = [
            [T([128, 2, sb], f32, f"pb{i}_{p}", space=PSUM) for p in range(2)]
            for i in range(2)
        ]
        if has_bias:
            bias_sb = T([1, H], wdt, "bias_sb")
            ones_sb = T([1, sb], wdt, "ones_sb")
            nc.sync.dma_start(bias_sb[:, :], bias_d[:, :])
            nc.gpsimd.memset(ones_sb[:, :], 1.0)

        for k in range(2):
            nc.sync.dma_start(wx_sb[:, k, :], wx_d[k * 128 : (k + 1) * 128, :])
            nc.sync.dma_start(wh_sb[:, k, :], wh_d[k * 128 : (k + 1) * 128, :])
        nc.sync.dma_start(h0_sb[:, :, :], h0_d[:, :, :])

        for blk in range(nblk):
            bi = blk % 2
            for k in range(2):
                nc.sync.dma_start(
                    xe_sb[bi][:, k, :],
                    xte_d[k * 128 : (k + 1) * 128, blk * sb : (blk + 1) * sb],
                )
                nc.sync.dma_start(
                    xo_sb[bi][:, k, :],
                    xto_d[k * 128 : (k + 1) * 128, blk * sb : (blk + 1) * sb],
                )

            # GEMM prefill: bank[p][:, m, n] = sum_d wx[d, m*128+.] * x[d, n]
            for p, xsb in ((0, xe_sb[bi]), (1, xo_sb[bi])):
                for m in range(2):
                    for k in range(2):
                        nc.tensor.matmul(
                            banks[bi][p][:, m, :],
                            wx_sb[:, k, m * 128 : (m + 1) * 128],
                            xsb[:, k, :],
                            start=(m == 0 and k == 0),
                            stop=False,
                            skip_group_check=True,
                        )
                    if has_bias:
                        nc.tensor.matmul(
                            banks[bi][p][:, m, :],
                            bias_sb[:, m * 128 : (m + 1) * 128],
                            ones_sb[:, :],
                            start=False,
                            stop=False,
                            skip_group_check=True,
                        )

            # serial scan: h_s = tanh(xp_s + Wh^T @ h_{s-1})  (all transposed).
            # ACT per output chunk m right after its two MMs, so next-step
            # MMs (which need chunk k of h) never stall the PE queue.
            for s in range(sb):
                p = s & 1
                sc = s >> 1
                bank = banks[bi][p]
                for m in range(2):
                    for k in range(2):
                        if s == 0 and blk == 0:
                            hp = h0_sb[:, k, :]
                        elif s == 0:
                            hp = ht_sb[1 - bi][:, k, 2 * sb - 2 : 2 * sb]
                        else:
                            hp = ht_sb[bi][:, k, 2 * s - 2 : 2 * s]
                        nc.tensor.matmul(
                            bank[:, m, 2 * sc : 2 * sc + 2],
                            wh_sb[:, k, m * 128 : (m + 1) * 128],
                            hp,
                            start=False,
                            stop=(m == 1 and k == 1 and s >= sb - 2),
                            skip_group_check=True,
                        )
                    nc.scalar.activation(
                        ht_sb[bi][:, m, 2 * s : 2 * s + 2],
                        bank[:, m, 2 * sc : 2 * sc + 2],
                        Tanh,
                        bias=0.0,
                        scale=1.0,
                    )

            nc.gpsimd.dma_start(
                yt_d[:, :, blk * 2 * sb : (blk + 1) * 2 * sb], ht_sb[bi][:, :, :]
            )

        for f in reversed(frees):
            f()

    nc.compile()
    return nc


def _get_nc(s_total=S, sb=SB, wdt_name="float32", has_bias=False):
    key = (s_total, sb, wdt_name, has_bias)
    if key not in _CACHE:
        _CACHE[key] = _build(s_total, sb, wdt_name, has_bias)
    return _CACHE[key]


LAST_EXEC_NS = None
LAST_RESULTS = None


def _np_dt(wdt_name):
    if wdt_name == "bfloat16":
        import ml_dtypes

        return ml_dtypes.bfloat16
    return np.float32


def kernel(inputs, state0, Wx, Wh, b, s_total=S, sb=SB, wdt_name="float32", trace=False):
    global LAST_EXEC_NS, LAST_RESULTS
    from concourse.bass_utils import run_bass_kernel_spmd

    inputs = np.asarray(inputs, dtype=np.float32)
    state0 = np.asarray(state0, dtype=np.float32)
    Wx = np.asarray(Wx, dtype=np.float32)
    Wh = np.asarray(Wh, dtype=np.float32)
    b = np.asarray(b, dtype=np.float32)
    has_bias = bool(np.any(b != 0))
    ndt = _np_dt(wdt_name)

    nc = _get_nc(s_total, sb, wdt_name, has_bias)

    in_maps = []
    wx_c = np.ascontiguousarray(Wx, dtype=ndt)
    wh_c = np.ascontiguousarray(Wh, dtype=ndt)
    for c in range(NCORES):
        xc = inputs[BLOCAL * c : BLOCAL * (c + 1), :s_total]  # [2, s, D]
        xt = np.transpose(xc, (2, 1, 0))  # [D, s, 2]
        xte = np.ascontiguousarray(xt[:, 0::2, :].reshape(D, s_total), dtype=ndt)
        xto = np.ascontiguousarray(xt[:, 1::2, :].reshape(D, s_total), dtype=ndt)
        h0 = np.ascontiguousarray(
            np.transpose(state0[BLOCAL * c : BLOCAL * (c + 1)].reshape(2, 2, 128), (2, 1, 0)),
            dtype=ndt,
        )  # [128, 2(chunk), 2(batch)]
        m = {"xte": xte, "xto": xto, "h0": h0, "wx": wx_c, "wh": wh_c}
        if has_bias:
            m["bias"] = np.ascontiguousarray(b.reshape(1, H), dtype=ndt)
        in_maps.append(m)

    res = run_bass_kernel_spmd(nc, in_maps, core_ids=list(range(NCORES)), trace=trace)
    LAST_EXEC_NS = res.exec_time_ns
    LAST_RESULTS = res

    outs = []
    for c in range(NCORES):
        yt = np.asarray(res.results[c]["yt"], dtype=np.float32)  # [128, 2, 2s]
        y = yt.reshape(128, 2, s_total, 2)  # (p, ch, s, b)
        y = np.transpose(y, (3, 2, 1, 0)).reshape(BLOCAL, s_total, H)
        outs.append(y)
    return np.ascontiguousarray(np.concatenate(outs, axis=0), dtype=np.float32)
